# revision 38
# baseline (speedup 1.0000x reference)
"""Trainium2 Bass kernel for the BEMv13 MoE-LoRA module (bf16, v9).

Computation (per token t, full problem):
  base  = x @ W_base.T + b_base
  w     = softmax(x @ W_router + b_router)        # E=2 experts
  out   = base + sum_e w_e * (x @ A_e.T) @ B_e.T * (alpha/rank)

Host-side algebra (exact): with w1 = 1 - w0,
  out = x @ W_eff.T + b_base + w0 * (x @ A_cat.T) @ Bd.T
  W_eff = W_base + scale*B1@A1   (folded on host, free)
  A_cat = [A0; A1]  [16, D],  Bd = scale*[B0, -B1]  [O, 16]
  w0    = sigmoid(x@(wr0-wr1) + (br0-br1))
so the on-chip routing chain is ONE sigmoid + ONE multiply.

Sharding: tokens (batch*seq = 16384) split across 8 NeuronCores; weights
replicated; no cross-core communication.

On-core algorithm (per core, 2048 tokens, all matmul operands bf16):
  - x pre-transposed AND pre-tiled on host: dram row block t holds the 16
    stationary lhsT tiles [k=128, tok=128] of token-tile t.
  - W^T pre-packed halves-major per half-slab [128(k), 1024(o)] bf16,
    resident in SBUF, streamed on the SP queue in consumption order.
  - Startup runs tiles 0..2 as a TRIPLE over half-O passes: 6 quarter
    matmuls per W half-slab (1.28us consumption) exceed the lumpy early
    DMA arrival (8 shared rings, ~3us completion lag each), so the PE
    stays nearly gap-free once started and the HAM clock gate warms to
    2.4GHz early and stays there.
  - PSUM start=True clears has_written for the WHOLE bank, so only two h
    accumulation groups may open concurrently (banks A and B). The third
    startup tile's h rides bank A start-less (skip_group_check): its
    first write to each element lands as an overwrite via the per-element
    has_written bits. Its slabs are spread over groups 2..14.
  - h_t stops a few k-groups early; the w0 chain (sigmoid -> mul -> DVE
    32x32 block transposes) then finishes well before the G-update.
    gt/bt are zero-padded to 128 contraction rows so the G matmuls keep
    the standard 128x128 stationary tile config (a 16-row stationary
    costs a ~100ns reconfig penalty).
  - The G-update for quarter j is interleaved right after that quarter's
    k=15 matmul, so drains (pure PSUM->SBUF bf16 copies, split DVE/ACT)
    start before the tile ends; b_base is added on the HOST.
  - Stores ride the Pool queue; the last tile drains+stores per quarter
    on 3 queues (last quarter split in two) so the end-of-kernel flush
    backlog is small.
"""

import numpy as np

P = 128
D = 2048
O = 2048
KT = D // P            # 16 k-slabs
TOK = 2048             # tokens per core
NT = TOK // P          # 16 token tiles
HN = 18                # 16 LoRA cols + 1 router-diff col + 1 pad
ER = 16                # E*R
HSTOP = KT - 4         # main-loop in-loop h stop (12); preblock covers 13..15
SCALE = 16.0 / 8.0
NCORES = 8
_CACHE = {}


def _build():
    import concourse.tile as tile
    from concourse import bacc, mybir

    f32 = mybir.dt.float32
    bf16 = mybir.dt.bfloat16

    nc = bacc.Bacc("TRN2", target_bir_lowering=False, debug=False)

    # xt: row block t = the 16 stationary lhsT tiles of token-tile t,
    # xt[t*P + p, k*P + j] = x[t*P + j, k*P + p]
    xt_d = nc.dram_tensor("xt", [TOK, D], bf16, kind="ExternalInput")
    # wt halves-major: wt[p, (hh*KT + k)*HO + c] = W_eff[hh*HO + c, k*P + p]
    wt_d = nc.dram_tensor("wt", [P, KT * O], bf16, kind="ExternalInput")
    aat_d = nc.dram_tensor("aat", [P, KT * HN], bf16, kind="ExternalInput")
    bt_d = nc.dram_tensor("bt", [P, O], bf16, kind="ExternalInput")
    brd_d = nc.dram_tensor("brd", [1, 1], f32, kind="ExternalInput")
    out_d = nc.dram_tensor("out", [TOK, O], bf16, kind="ExternalOutput")

    HO = O // 2

    with tile.TileContext(nc) as tc:
        with (
            tc.tile_pool(name="res", bufs=1) as res,
            tc.tile_pool(name="obuf", bufs=3) as obuf,
            tc.tile_pool(name="xpool", bufs=4) as xpool,
            tc.tile_pool(name="small", bufs=3) as small,
            tc.tile_pool(name="psA", bufs=6, space="PSUM") as psA,
            tc.tile_pool(name="psH", bufs=2, space="PSUM") as psH,
        ):
            # --- W^T stream: SP queue only, half-slabs in consumption
            # order. The first ~8 DMA ring slots are precious: only W and
            # x may occupy them.
            wt_b = res.tile([P, KT * O], bf16, tag="wt_b")

            def wslab(hh, k):  # [P, HO] half-slab view in SBUF
                i = hh * KT + k
                return wt_b[:, i * HO:(i + 1) * HO]

            for i in range(2 * KT):
                nc.sync.dma_start(wt_b[:, i * HO:(i + 1) * HO],
                                  wt_d[:, i * HO:(i + 1) * HO])

            # x tok-tile loads + small constants on the ACT HWDGE queue.
            x_tiles = [None] * NT

            def load_x(t, chunks=1):
                x_tiles[t] = xpool.tile([P, D], bf16, tag="x", name=f"x_{t}")
                cw = D // chunks
                for cc in range(chunks):
                    nc.scalar.dma_start(
                        x_tiles[t][:, cc * cw:(cc + 1) * cw],
                        xt_d[t * P:(t + 1) * P, cc * cw:(cc + 1) * cw])

            # x0 cols 0:1024, x1, x2, x0 tail, then the small constants
            # (the triple phase zippers t0/t1/t2, so all three want data
            # early; the scheduler runs whatever arrived first).
            for t in range(3):
                x_tiles[t] = xpool.tile([P, D], bf16, tag="x", name=f"x_{t}")
            nc.scalar.dma_start(x_tiles[0][:, 0:512], xt_d[0:P, 0:512])
            nc.scalar.dma_start(x_tiles[0][:, 512:1024], xt_d[0:P, 512:1024])
            for t in range(1, 3):
                for cc in range(2):
                    nc.scalar.dma_start(
                        x_tiles[t][:, cc * 1024:(cc + 1) * 1024],
                        xt_d[t * P:(t + 1) * P, cc * 1024:(cc + 1) * 1024])
            nc.scalar.dma_start(x_tiles[0][:, 1024:1536], xt_d[0:P, 1024:1536])
            nc.scalar.dma_start(x_tiles[0][:, 1536:2048], xt_d[0:P, 1536:2048])

            aat_b = res.tile([P, KT * HN], bf16, tag="aat_b")
            nc.scalar.dma_start(aat_b[:], aat_d[:])
            bt_b = res.tile([P, O], bf16, tag="bt_b")
            nc.scalar.dma_start(bt_b[:], bt_d[:])
            # router bias diff (b_r0 - b_r1), partition-broadcast
            brd128 = res.tile([P, 1], f32, tag="brd128")
            nc.scalar.dma_start(brd128[:], brd_d[:].broadcast_to((P, 1)))

            def lhs(t, k):
                return x_tiles[t][:, k * P:(k + 1) * P]

            # routing weight + scaled-H transpose; returns gt [P, P] bf16
            # (partitions 16..127 zero: standard 128-row stationary size).
            # h lives at column `base` of PSUM tile `ht`.
            def make_gt(t, ht, base=0):
                w0s = small.tile([P, 1], f32, tag="w0s", name=f"w0s_{t}")
                nc.scalar.activation(w0s[:], ht[:, base + ER:base + ER + 1],
                                     mybir.ActivationFunctionType.Sigmoid,
                                     bias=brd128[:, 0:1], scale=1.0)
                g = small.tile([P, 32], bf16, tag="g", name=f"g_{t}")
                nc.vector.memset(g[:, ER:32], 0.0)
                nc.vector.tensor_scalar_mul(g[:, 0:ER],
                                            ht[:, base:base + ER], w0s[:])
                gt = small.tile([P, P], bf16, tag="gt", name=f"gt_{t}")
                nc.vector.memset(gt[32:64, :], 0.0)
                nc.vector.memset(gt[64:P, :], 0.0)
                for r in range(4):
                    nc.vector.transpose(gt[0:32, r * 32:(r + 1) * 32],
                                        g[r * 32:(r + 1) * 32, 0:32])
                return gt

            def h_matmul(ht, base, t, k, start, stop, skip=False):
                nc.tensor.matmul(ht[:, base:base + HN], lhs(t, k),
                                 aat_b[:, k * HN:(k + 1) * HN],
                                 start=start, stop=stop,
                                 skip_group_check=skip)

            # h pre-block: open the accumulation with the slabs above hstop
            def h_preblock(ht, base, t, hstop):
                for k in range(hstop + 1, KT):
                    h_matmul(ht, base, t, k, start=(k == hstop + 1),
                             stop=False)

            # drain quarter jj of tile t as a pure copy (bias on host);
            # even jj on DVE, odd on ACT.
            def drain(t, acc, jj, cols=None, eng=None):
                c0, c1 = (jj * 512, (jj + 1) * 512) if cols is None else cols
                dst = out_tiles[t][:, c0:c1]
                a = acc[:, c0 - jj * 512:c1 - jj * 512] if cols else acc[:]
                if (jj % 2 == 0) if eng is None else (eng == "dve"):
                    nc.vector.tensor_copy(dst, a)
                else:
                    nc.scalar.copy(dst, a)

            def store(t, half):
                nc.gpsimd.dma_start(
                    out_d[t * P:(t + 1) * P, half * HO:(half + 1) * HO],
                    out_tiles[t][:, half * HO:(half + 1) * HO])

            out_tiles = [None] * NT

            # =========== startup: tiles 0..2 as a TRIPLE, half-O passes ====
            # h0 -> bank A cols 0:18 (start=True via preblock), h1 -> bank
            # B (own start). h2 -> bank A cols 32:50 WITHOUT any start:
            # PSUM start=True clears has_written for the WHOLE bank, so a
            # third concurrent group must ride an existing bank start-less
            # (per-element has_written makes its first write an overwrite).
            # h2's slabs are spread over groups 2..14, stop at 14.
            HS3 = [10, 11]           # staggered h stops for t=0,1
            hA = psH.tile([P, 64], f32, tag="h", name="hA")
            hB = psH.tile([P, HN], f32, tag="h", name="hB")
            t2_groups = [[] for _ in range(KT)]
            for s in range(KT):
                t2_groups[2 + (s * 13) // 16].append(s)
            gts = [None] * 3
            for t in range(3):
                out_tiles[t] = obuf.tile([P, O], bf16, tag="obuf",
                                         name=f"out_{t}")

            # tile entry stagger: the PE queue is strictly in-order, so a
            # tile whose x has not landed would stall everything behind it.
            # t1 enters 3 k-groups late, t2 5 late, matching x1/x2 arrival.
            LAG = [0, 0, 0]
            for hh in range(2):
                accs3 = [[psA.tile([P, 512], f32, tag="acc",
                                   name=f"acc_{t}_{hh}_{j}")
                          for j in range(2)] for t in range(3)]
                if hh == 0:
                    h_preblock(hA, 0, 0, HS3[0])
                    h_preblock(hB, 0, 1, HS3[1])
                for s in range(KT + LAG[2]):
                    for t in range(3):
                        k = s - LAG[t]
                        if not (0 <= k < KT):
                            continue
                        if hh == 0 and t < 2 and k <= HS3[t]:
                            h_matmul((hA, hB)[t], 0, t, k, start=False,
                                     stop=(k == HS3[t]))
                        if hh == 0 and t == 2:
                            for sl in t2_groups[k]:
                                h_matmul(hA, 32, 2, sl, start=False,
                                         stop=(sl == KT - 1), skip=True)
                        for j in range(2):
                            nc.tensor.matmul(
                                accs3[t][j][:], lhs(t, k),
                                wslab(hh, k)[:, j * 512:(j + 1) * 512],
                                start=(k == 0), stop=False)
                            if k == KT - 1:
                                nc.tensor.matmul(
                                    accs3[t][j][:], gts[t][:],
                                    bt_b[:, hh * HO + j * 512:
                                         hh * HO + (j + 1) * 512],
                                    start=False, stop=True)
                        if hh == 0 and t < 2 and k == HS3[t]:
                            gts[t] = make_gt(t, (hA, hB)[t], 0)
                        if hh == 0 and t == 2 and k == KT - 2:
                            gts[2] = make_gt(2, hA, 32)
                        if k == KT - 1:
                            for j in range(2):
                                drain(t, accs3[t][j], 2 * hh + j)
                            store(t, hh)

            load_x(3)
            load_x(4)

            # =========== main loop: tiles 3..15, one tile at a time ========
            for t in range(3, NT):
                out_tiles[t] = obuf.tile([P, O], bf16, tag="obuf",
                                         name=f"out_{t}")
                accs = [psA.tile([P, 512], f32, tag="acc", name=f"acc_{t}_{j}")
                        for j in range(4)]
                h = psH.tile([P, HN], f32, tag="h", name=f"h_{t}")
                h_preblock(h, 0, t, HSTOP)
                gt = None
                for k in range(KT):
                    if k <= HSTOP:
                        h_matmul(h, 0, t, k, start=False, stop=(k == HSTOP))
                    for j in range(4):
                        nc.tensor.matmul(
                            accs[j][:], lhs(t, k),
                            wslab(j // 2, k)[:, (j % 2) * 512:
                                             (j % 2 + 1) * 512],
                            start=(k == 0), stop=False)
                        # G-update interleaved per quarter into the last
                        # k-group: quarter j can drain ~3 matmuls earlier.
                        if k == KT - 1:
                            nc.tensor.matmul(accs[j][:], gt[:],
                                             bt_b[:, j * 512:(j + 1) * 512],
                                             start=False, stop=True)
                    if k == HSTOP:
                        gt = make_gt(t, h, 0)
                    if k == 8 and t + 2 < NT:
                        load_x(t + 2, chunks=4)
                if t == NT - 1:
                    # final tile: per-quarter drains + stores on 3 queues;
                    # last quarter as two 256-col halves on two queues.
                    for j, q in zip(range(3),
                                    (nc.gpsimd, nc.sync, nc.scalar)):
                        drain(t, accs[j], j)
                        q.dma_start(
                            out_d[t * P:(t + 1) * P, j * 512:(j + 1) * 512],
                            out_tiles[t][:, j * 512:(j + 1) * 512])
                    drain(t, accs[3], 3, cols=(1536, 1792), eng="act")
                    nc.sync.dma_start(out_d[t * P:(t + 1) * P, 1536:1792],
                                      out_tiles[t][:, 1536:1792])
                    drain(t, accs[3], 3, cols=(1792, 2048), eng="dve")
                    nc.scalar.dma_start(out_d[t * P:(t + 1) * P, 1792:2048],
                                        out_tiles[t][:, 1792:2048])
                else:
                    drain(t, accs[0], 0)
                    drain(t, accs[1], 1)
                    store(t, 0)
                    drain(t, accs[2], 2)
                    drain(t, accs[3], 3)
                    store(t, 1)

    nc.compile()
    return nc


def _prep_host(x, W_base, b_base, A, B, W_router, b_router):
    """Host-side layout prep + sharding. Returns per-core input maps."""
    import ml_dtypes
    bf16 = ml_dtypes.bfloat16

    A = np.asarray(A, dtype=np.float32)
    B = np.asarray(B, dtype=np.float32)
    wr = np.asarray(W_router, dtype=np.float32)

    x_flat = np.ascontiguousarray(x, dtype=np.float32).reshape(-1, D)
    # xt[t*P + p, k*P + j] = x[t*P + j, k*P + p], per core
    NTOT = x_flat.shape[0] // P
    xt_all = np.ascontiguousarray(
        x_flat.reshape(NTOT, P, KT, P).transpose(0, 3, 2, 1)
    ).reshape(NTOT * P, D).astype(bf16)

    # W_eff = W_base + scale*B1@A1, folded on host.
    # Packed halves-major: wt_p[p, (hh*KT + k)*HO + c] = wt[k*P+p, hh*HO+c]
    HO = O // 2
    w_eff = np.asarray(W_base, dtype=np.float32) + SCALE * (B[1] @ A[1])
    wt = w_eff.T                                                    # [D, O]
    wt_p = np.ascontiguousarray(
        wt.reshape(KT, P, 2, HO).transpose(1, 2, 0, 3).reshape(P, KT * O)
    ).astype(bf16)

    a_cat = A.reshape(ER, D)                                        # [16, D]
    aat = np.zeros((D, HN), dtype=np.float32)
    aat[:, :ER] = a_cat.T
    aat[:, ER] = wr[:, 0] - wr[:, 1]
    aat_p = np.ascontiguousarray(
        aat.reshape(KT, P, HN).transpose(1, 0, 2).reshape(P, KT * HN)
    ).astype(bf16)

    b_d = np.concatenate([B[0], -B[1]], axis=1)                     # [O, 16]
    bt = np.zeros((P, O), dtype=np.float32)                         # [128, O]
    bt[:ER] = b_d.T * SCALE
    bt = np.ascontiguousarray(bt).astype(bf16)
    dlb = np.float32(b_router[0]) - np.float32(b_router[1])
    brd = np.array([[dlb]], dtype=np.float32)

    in_maps = []
    for c in range(NCORES):
        in_maps.append({
            "xt": xt_all[c * TOK:(c + 1) * TOK],
            "wt": wt_p,
            "aat": aat_p,
            "bt": bt,
            "brd": brd,
        })
    return in_maps


def kernel(x, W_base, b_base, A, B, W_router, b_router):
    from concourse import bass_utils

    if "nc" not in _CACHE:
        _CACHE["nc"] = _build()
    nc = _CACHE["nc"]

    in_maps = _prep_host(x, W_base, b_base, A, B, W_router, b_router)
    res = None
    for attempt in range(3):
        try:
            res = bass_utils.run_bass_kernel_spmd(
                nc, in_maps, core_ids=list(range(NCORES)))
            break
        except Exception:
            # rare transient NRT_EXEC_UNIT_UNRECOVERABLE observed once;
            # the same NEFF runs fine on retry
            if attempt == 2:
                raise
    out = np.concatenate([res.results[c]["out"] for c in range(NCORES)], axis=0)
    out = out.astype(np.float32) + np.asarray(b_base, dtype=np.float32)
    return out.reshape(np.asarray(x).shape[0], -1, O)


# revision 40
# speedup vs baseline: 1.0006x; 1.0006x over previous
"""Trainium2 Bass kernel for the BEMv13 MoE-LoRA module (bf16, v9).

Computation (per token t, full problem):
  base  = x @ W_base.T + b_base
  w     = softmax(x @ W_router + b_router)        # E=2 experts
  out   = base + sum_e w_e * (x @ A_e.T) @ B_e.T * (alpha/rank)

Host-side algebra (exact): with w1 = 1 - w0,
  out = x @ W_eff.T + b_base + w0 * (x @ A_cat.T) @ Bd.T
  W_eff = W_base + scale*B1@A1   (folded on host, free)
  A_cat = [A0; A1]  [16, D],  Bd = scale*[B0, -B1]  [O, 16]
  w0    = sigmoid(x@(wr0-wr1) + (br0-br1))
so the on-chip routing chain is ONE sigmoid + ONE multiply.

Sharding: tokens (batch*seq = 16384) split across 8 NeuronCores; weights
replicated; no cross-core communication.

On-core algorithm (per core, 2048 tokens, all matmul operands bf16):
  - x pre-transposed AND pre-tiled on host: dram row block t holds the 16
    stationary lhsT tiles [k=128, tok=128] of token-tile t.
  - W^T pre-packed halves-major per half-slab [128(k), 1024(o)] bf16,
    resident in SBUF, streamed on the SP queue in consumption order.
  - Startup runs tiles 0..2 as a TRIPLE over half-O passes: 6 quarter
    matmuls per W half-slab (1.28us consumption) exceed the lumpy early
    DMA arrival (8 shared rings, ~3us completion lag each), so the PE
    stays nearly gap-free once started and the HAM clock gate warms to
    2.4GHz early and stays there.
  - PSUM start=True clears has_written for the WHOLE bank, so only two h
    accumulation groups may open concurrently (banks A and B). The third
    startup tile's h rides bank A start-less (skip_group_check): its
    first write to each element lands as an overwrite via the per-element
    has_written bits. Its slabs are spread over groups 2..14.
  - h_t stops a few k-groups early; the w0 chain (sigmoid -> mul -> DVE
    32x32 block transposes) then finishes well before the G-update.
    gt/bt are zero-padded to 128 contraction rows so the G matmuls keep
    the standard 128x128 stationary tile config (a 16-row stationary
    costs a ~100ns reconfig penalty).
  - The G-update for quarter j is interleaved right after that quarter's
    k=15 matmul, so drains (pure PSUM->SBUF bf16 copies, split DVE/ACT)
    start before the tile ends; b_base is added on the HOST.
  - Stores ride the Pool queue; the last tile drains+stores per quarter
    on 3 queues (last quarter split in two) so the end-of-kernel flush
    backlog is small.
"""

import numpy as np

P = 128
D = 2048
O = 2048
KT = D // P            # 16 k-slabs
TOK = 2048             # tokens per core
NT = TOK // P          # 16 token tiles
HN = 18                # 16 LoRA cols + 1 router-diff col + 1 pad
ER = 16                # E*R
HSTOP = KT - 4         # main-loop in-loop h stop (12); preblock covers 13..15
SCALE = 16.0 / 8.0
NCORES = 8
_CACHE = {}


def _build():
    import concourse.tile as tile
    from concourse import bacc, mybir

    f32 = mybir.dt.float32
    bf16 = mybir.dt.bfloat16

    nc = bacc.Bacc("TRN2", target_bir_lowering=False, debug=False)

    # xt: row block t = the 16 stationary lhsT tiles of token-tile t,
    # xt[t*P + p, k*P + j] = x[t*P + j, k*P + p]
    xt_d = nc.dram_tensor("xt", [TOK, D], bf16, kind="ExternalInput")
    # wt halves-major: wt[p, (hh*KT + k)*HO + c] = W_eff[hh*HO + c, k*P + p]
    wt_d = nc.dram_tensor("wt", [P, KT * O], bf16, kind="ExternalInput")
    aat_d = nc.dram_tensor("aat", [P, KT * HN], bf16, kind="ExternalInput")
    bt_d = nc.dram_tensor("bt", [P, O], bf16, kind="ExternalInput")
    brd_d = nc.dram_tensor("brd", [1, 1], f32, kind="ExternalInput")
    out_d = nc.dram_tensor("out", [TOK, O], bf16, kind="ExternalOutput")

    HO = O // 2

    with tile.TileContext(nc) as tc:
        with (
            tc.tile_pool(name="res", bufs=1) as res,
            tc.tile_pool(name="obuf", bufs=3) as obuf,
            tc.tile_pool(name="xpool", bufs=4) as xpool,
            tc.tile_pool(name="small", bufs=3) as small,
            tc.tile_pool(name="psA", bufs=6, space="PSUM") as psA,
            tc.tile_pool(name="psH", bufs=2, space="PSUM") as psH,
        ):
            # --- W^T stream: SP queue only, half-slabs in consumption
            # order. The first ~8 DMA ring slots are precious: only W and
            # x may occupy them.
            wt_b = res.tile([P, KT * O], bf16, tag="wt_b")

            def wslab(hh, k):  # [P, HO] half-slab view in SBUF
                i = hh * KT + k
                return wt_b[:, i * HO:(i + 1) * HO]

            for i in range(2 * KT):
                nc.sync.dma_start(wt_b[:, i * HO:(i + 1) * HO],
                                  wt_d[:, i * HO:(i + 1) * HO])

            # x tok-tile loads + small constants on the ACT HWDGE queue.
            x_tiles = [None] * NT

            def load_x(t, chunks=1):
                x_tiles[t] = xpool.tile([P, D], bf16, tag="x", name=f"x_{t}")
                cw = D // chunks
                for cc in range(chunks):
                    nc.scalar.dma_start(
                        x_tiles[t][:, cc * cw:(cc + 1) * cw],
                        xt_d[t * P:(t + 1) * P, cc * cw:(cc + 1) * cw])

            # x0 cols 0:1024, x1, x2, x0 tail, then the small constants
            # (the triple phase zippers t0/t1/t2, so all three want data
            # early; the scheduler runs whatever arrived first).
            for t in range(3):
                x_tiles[t] = xpool.tile([P, D], bf16, tag="x", name=f"x_{t}")
            nc.scalar.dma_start(x_tiles[0][:, 0:512], xt_d[0:P, 0:512])
            nc.scalar.dma_start(x_tiles[0][:, 512:1024], xt_d[0:P, 512:1024])
            for t in range(1, 3):
                for cc in range(2):
                    nc.scalar.dma_start(
                        x_tiles[t][:, cc * 1024:(cc + 1) * 1024],
                        xt_d[t * P:(t + 1) * P, cc * 1024:(cc + 1) * 1024])
            nc.scalar.dma_start(x_tiles[0][:, 1024:1536], xt_d[0:P, 1024:1536])
            nc.scalar.dma_start(x_tiles[0][:, 1536:2048], xt_d[0:P, 1536:2048])

            aat_b = res.tile([P, KT * HN], bf16, tag="aat_b")
            nc.scalar.dma_start(aat_b[:], aat_d[:])
            bt_b = res.tile([P, O], bf16, tag="bt_b")
            nc.scalar.dma_start(bt_b[:], bt_d[:])
            # router bias diff (b_r0 - b_r1), partition-broadcast
            brd128 = res.tile([P, 1], f32, tag="brd128")
            nc.scalar.dma_start(brd128[:], brd_d[:].broadcast_to((P, 1)))

            def lhs(t, k):
                return x_tiles[t][:, k * P:(k + 1) * P]

            # routing weight + scaled-H transpose; returns gt [P, P] bf16
            # (partitions 16..127 zero: standard 128-row stationary size).
            # h lives at column `base` of PSUM tile `ht`.
            def make_gt(t, ht, base=0):
                w0s = small.tile([P, 1], f32, tag="w0s", name=f"w0s_{t}")
                nc.scalar.activation(w0s[:], ht[:, base + ER:base + ER + 1],
                                     mybir.ActivationFunctionType.Sigmoid,
                                     bias=brd128[:, 0:1], scale=1.0)
                g = small.tile([P, 32], bf16, tag="g", name=f"g_{t}")
                nc.vector.memset(g[:, ER:32], 0.0)
                nc.vector.tensor_scalar_mul(g[:, 0:ER],
                                            ht[:, base:base + ER], w0s[:])
                gt = small.tile([P, P], bf16, tag="gt", name=f"gt_{t}")
                nc.vector.memset(gt[32:64, :], 0.0)
                nc.vector.memset(gt[64:P, :], 0.0)
                for r in range(4):
                    nc.vector.transpose(gt[0:32, r * 32:(r + 1) * 32],
                                        g[r * 32:(r + 1) * 32, 0:32])
                return gt

            def h_matmul(ht, base, t, k, start, stop, skip=False):
                nc.tensor.matmul(ht[:, base:base + HN], lhs(t, k),
                                 aat_b[:, k * HN:(k + 1) * HN],
                                 start=start, stop=stop,
                                 skip_group_check=skip)

            # h pre-block: open the accumulation with the slabs above hstop
            def h_preblock(ht, base, t, hstop):
                for k in range(hstop + 1, KT):
                    h_matmul(ht, base, t, k, start=(k == hstop + 1),
                             stop=False)

            # drain quarter jj of tile t as a pure copy (bias on host);
            # even jj on DVE, odd on ACT.
            def drain(t, acc, jj, cols=None, eng=None):
                c0, c1 = (jj * 512, (jj + 1) * 512) if cols is None else cols
                dst = out_tiles[t][:, c0:c1]
                a = acc[:, c0 - jj * 512:c1 - jj * 512] if cols else acc[:]
                if (jj % 2 == 0) if eng is None else (eng == "dve"):
                    nc.vector.tensor_copy(dst, a)
                else:
                    nc.scalar.copy(dst, a)

            def store(t, half):
                nc.gpsimd.dma_start(
                    out_d[t * P:(t + 1) * P, half * HO:(half + 1) * HO],
                    out_tiles[t][:, half * HO:(half + 1) * HO])

            out_tiles = [None] * NT

            # =========== startup: tiles 0..2 as a TRIPLE, half-O passes ====
            # h0 -> bank A cols 0:18 (start=True via preblock), h1 -> bank
            # B (own start). h2 -> bank A cols 32:50 WITHOUT any start:
            # PSUM start=True clears has_written for the WHOLE bank, so a
            # third concurrent group must ride an existing bank start-less
            # (per-element has_written makes its first write an overwrite).
            # h2's slabs are spread over groups 2..14, stop at 14.
            HS3 = [10, 11]           # staggered h stops for t=0,1
            hA = psH.tile([P, 64], f32, tag="h", name="hA")
            hB = psH.tile([P, HN], f32, tag="h", name="hB")
            t2_groups = [[] for _ in range(KT)]
            for s in range(KT):
                t2_groups[2 + (s * 13) // 16].append(s)
            gts = [None] * 3
            for t in range(3):
                out_tiles[t] = obuf.tile([P, O], bf16, tag="obuf",
                                         name=f"out_{t}")

            # tile entry stagger: the PE queue is strictly in-order, so a
            # tile whose x has not landed would stall everything behind it.
            # t1 enters 3 k-groups late, t2 5 late, matching x1/x2 arrival.
            LAG = [0, 0, 0]
            for hh in range(2):
                accs3 = [[psA.tile([P, 512], f32, tag="acc",
                                   name=f"acc_{t}_{hh}_{j}")
                          for j in range(2)] for t in range(3)]
                if hh == 0:
                    h_preblock(hA, 0, 0, HS3[0])
                    h_preblock(hB, 0, 1, HS3[1])
                for s in range(KT + LAG[2]):
                    for t in range(3):
                        k = s - LAG[t]
                        if not (0 <= k < KT):
                            continue
                        if hh == 0 and t < 2 and k <= HS3[t]:
                            h_matmul((hA, hB)[t], 0, t, k, start=False,
                                     stop=(k == HS3[t]))
                        if hh == 0 and t == 2:
                            for sl in t2_groups[k]:
                                h_matmul(hA, 32, 2, sl, start=False,
                                         stop=(sl == KT - 1), skip=True)
                        for j in range(2):
                            nc.tensor.matmul(
                                accs3[t][j][:], lhs(t, k),
                                wslab(hh, k)[:, j * 512:(j + 1) * 512],
                                start=(k == 0), stop=False)
                            if k == KT - 1:
                                nc.tensor.matmul(
                                    accs3[t][j][:], gts[t][:],
                                    bt_b[:, hh * HO + j * 512:
                                         hh * HO + (j + 1) * 512],
                                    start=False, stop=True)
                        if hh == 0 and t < 2 and k == HS3[t]:
                            gts[t] = make_gt(t, (hA, hB)[t], 0)
                        if hh == 0 and t == 2 and k == KT - 2:
                            gts[2] = make_gt(2, hA, 32)
                        if k == KT - 1:
                            for j in range(2):
                                drain(t, accs3[t][j], 2 * hh + j)
                            store(t, hh)

            load_x(3)
            load_x(4)

            # =========== main loop: tiles 3..15, one tile at a time ========
            for t in range(3, NT):
                out_tiles[t] = obuf.tile([P, O], bf16, tag="obuf",
                                         name=f"out_{t}")
                accs = [psA.tile([P, 512], f32, tag="acc", name=f"acc_{t}_{j}")
                        for j in range(4)]
                h = psH.tile([P, HN], f32, tag="h", name=f"h_{t}")
                h_preblock(h, 0, t, HSTOP)
                gt = None
                for k in range(KT):
                    if k <= HSTOP:
                        h_matmul(h, 0, t, k, start=False, stop=(k == HSTOP))
                    for j in range(4):
                        nc.tensor.matmul(
                            accs[j][:], lhs(t, k),
                            wslab(j // 2, k)[:, (j % 2) * 512:
                                             (j % 2 + 1) * 512],
                            start=(k == 0), stop=False)
                        # G-update interleaved per quarter into the last
                        # k-group: quarter j can drain ~3 matmuls earlier.
                        if k == KT - 1:
                            nc.tensor.matmul(accs[j][:], gt[:],
                                             bt_b[:, j * 512:(j + 1) * 512],
                                             start=False, stop=True)
                    if k == HSTOP:
                        gt = make_gt(t, h, 0)
                    if k == 8 and t + 2 < NT:
                        load_x(t + 2, chunks=4)
                if t == NT - 1:
                    # final tile: per-quarter drains + stores on 3 queues;
                    # last quarter as two 256-col halves on two queues.
                    for j, q in zip(range(3),
                                    (nc.gpsimd, nc.sync, nc.scalar)):
                        drain(t, accs[j], j)
                        q.dma_start(
                            out_d[t * P:(t + 1) * P, j * 512:(j + 1) * 512],
                            out_tiles[t][:, j * 512:(j + 1) * 512])
                    drain(t, accs[3], 3, cols=(1536, 1792), eng="act")
                    nc.sync.dma_start(out_d[t * P:(t + 1) * P, 1536:1792],
                                      out_tiles[t][:, 1536:1792])
                    drain(t, accs[3], 3, cols=(1792, 2048), eng="dve")
                    nc.scalar.dma_start(out_d[t * P:(t + 1) * P, 1792:2048],
                                        out_tiles[t][:, 1792:2048])
                else:
                    drain(t, accs[0], 0)
                    drain(t, accs[1], 1)
                    store(t, 0)
                    drain(t, accs[2], 2)
                    drain(t, accs[3], 3)
                    store(t, 1)

    nc.compile()
    return nc


def _prep_host(x, W_base, b_base, A, B, W_router, b_router):
    """Host-side layout prep + sharding. Returns per-core input maps."""
    import ml_dtypes
    bf16 = ml_dtypes.bfloat16

    A = np.asarray(A, dtype=np.float32)
    B = np.asarray(B, dtype=np.float32)
    wr = np.asarray(W_router, dtype=np.float32)

    x_flat = np.ascontiguousarray(x, dtype=np.float32).reshape(-1, D)
    # xt[t*P + p, k*P + j] = x[t*P + j, k*P + p], per core
    NTOT = x_flat.shape[0] // P
    xt_all = np.ascontiguousarray(
        x_flat.reshape(NTOT, P, KT, P).transpose(0, 3, 2, 1)
    ).reshape(NTOT * P, D).astype(bf16)

    # W_eff = W_base + scale*B1@A1, folded on host.
    # Packed halves-major: wt_p[p, (hh*KT + k)*HO + c] = wt[k*P+p, hh*HO+c]
    HO = O // 2
    w_eff = np.asarray(W_base, dtype=np.float32) + SCALE * (B[1] @ A[1])
    wt = w_eff.T                                                    # [D, O]
    wt_p = np.ascontiguousarray(
        wt.reshape(KT, P, 2, HO).transpose(1, 2, 0, 3).reshape(P, KT * O)
    ).astype(bf16)

    a_cat = A.reshape(ER, D)                                        # [16, D]
    aat = np.zeros((D, HN), dtype=np.float32)
    aat[:, :ER] = a_cat.T
    aat[:, ER] = wr[:, 0] - wr[:, 1]
    aat_p = np.ascontiguousarray(
        aat.reshape(KT, P, HN).transpose(1, 0, 2).reshape(P, KT * HN)
    ).astype(bf16)

    b_d = np.concatenate([B[0], -B[1]], axis=1)                     # [O, 16]
    bt = np.zeros((P, O), dtype=np.float32)                         # [128, O]
    bt[:ER] = b_d.T * SCALE
    bt = np.ascontiguousarray(bt).astype(bf16)
    dlb = np.float32(b_router[0]) - np.float32(b_router[1])
    brd = np.array([[dlb]], dtype=np.float32)

    in_maps = []
    for c in range(NCORES):
        in_maps.append({
            "xt": xt_all[c * TOK:(c + 1) * TOK],
            "wt": wt_p,
            "aat": aat_p,
            "bt": bt,
            "brd": brd,
        })
    return in_maps


def kernel(x, W_base, b_base, A, B, W_router, b_router):
    from concourse import bass_utils

    if "nc" not in _CACHE:
        _CACHE["nc"] = _build()
    nc = _CACHE["nc"]

    in_maps = _prep_host(x, W_base, b_base, A, B, W_router, b_router)
    res = None
    for attempt in range(3):
        try:
            res = bass_utils.run_bass_kernel_spmd(
                nc, in_maps, core_ids=list(range(NCORES)))
            break
        except Exception:
            # rare transient NRT_EXEC_UNIT_UNRECOVERABLE observed once;
            # the same NEFF runs fine on retry
            if attempt == 2:
                raise
    out = np.concatenate([res.results[c]["out"] for c in range(NCORES)], axis=0)
    out = out.astype(np.float32) + np.asarray(b_base, dtype=np.float32)
    return out.reshape(np.asarray(x).shape[0], -1, O)
